# revision 17
# baseline (speedup 1.0000x reference)
"""Causal GQA self-attention on 8 Trainium2 NeuronCores.

Sharding: data-parallel over batch (4) x tensor-parallel over heads (2 halves
of 14 heads each, KV heads replicated for the shared GQA group). Each core
computes a partial output (its heads' contribution through the row-parallel
out-projection); the host sums the two partials per batch element.

Per-core head assignment is chosen so every core sees an identical local
structure (local heads 0..13, local kv-groups 0..3, quad q <-> group q):
  half 0: global heads [0..11, 24, 25],  kv heads [0, 1, 2, 6]
  half 1: global heads [12..23, 26, 27], kv heads [3, 4, 5, 6]
The host permutes weight columns/rows into this local order.

Kernel layout strategy (all SBUF tensors [128 partitions, free...]):
  xT  [128, 7, 2048] : x^T (C on partitions) via PE transpose
  QT  [128, 4, 2048] : Q^T, local head h at (partitions 32*(h%4), chunk h//4)
  KT  [128, 4, 2048] : K^T per local group, replicated on all 4 row slots
  V   [128, 16, 128] : V (kpos on partitions)
  AOT [128, 4, 2048] : attention output transposed (head dims on partitions)
Scores are computed transposed S^T[kpos, q] with 4 row-tiled (tile_position)
K=32 matmuls per quad; exp on ScalarE (PSUM->SBUF, scale folded in); P^T then
feeds col-tiled AV and Z(=sum) matmuls accumulating over kpos chunks; final
out-projection consumes AOT directly as the stationary operand.
"""

import sys

sys.path.insert(0, "/opt/trn_rl_repo")

from contextlib import ExitStack

import numpy as np

import concourse.bass as bass
import concourse.mybir as mybir
import concourse.tile as tile
from concourse import bacc
from concourse.bass import ts
from concourse.bass_utils import run_bass_kernel_spmd

F32 = mybir.dt.float32
F32R = mybir.dt.float32r
EXP = mybir.ActivationFunctionType.Exp
P = 128
T, C = 2048, 896
D = 32
HL = 14  # local heads per core
GL = 4  # local kv groups per core
DH = HL * D  # 448
DKV = GL * D  # 128
SCALE = 1.0 / float(np.sqrt(D))

HEADS_HALF = [
    list(range(0, 12)) + [24, 25],
    list(range(12, 24)) + [26, 27],
]
KV_HALF = [[0, 1, 2, 6], [3, 4, 5, 6]]


def _trace(tc, d):
    nc = tc.nc
    with ExitStack() as ctx:
        const = ctx.enter_context(tc.tile_pool(name="const", bufs=1))
        ident = const.tile([P, P], F32)
        nc.sync.dma_start(ident[:], d["ident"][:])
        maskb = const.tile([P, P], F32)
        nc.sync.dma_start(maskb[:], d["mask"][:])
        identr = const.tile([P, P], F32R)
        nc.sync.dma_start(identr[:], d["identr"][:])

        persist = ctx.enter_context(tc.tile_pool(name="persist", bufs=1))
        QT = persist.tile([P, 4, T], F32R, tag="QT")
        KT = persist.tile([P, 4, T], F32R, tag="KT")
        V = persist.tile([P, 16, GL, 64], F32R, tag="V")

        nc.sync.dma_start(
            V[:, :, :, D:64],
            d["vones"].rearrange("p (a b c) -> p a b c", a=16, b=GL),
        )

        with tc.tile_pool(name="ph01", bufs=1) as ph01:
            xT = ph01.tile([P, 7, T], F32R, tag="xT")
            # ------------- phase 0: x -> xT (PE transpose) -------------
            with tc.tile_pool(name="xraw", bufs=8) as xraw, \
                 tc.tile_pool(name="pst", bufs=2, space="PSUM") as pst:
                xv = d["x"].rearrange("(to ti) c -> ti to c", ti=P)
                for tcg in range(4):
                    xt4 = []
                    for k in range(4):
                        xtile = xraw.tile([P, C], F32, tag="xtile")
                        nc.sync.dma_start(xtile[:], xv[:, 4 * tcg + k, :])
                        xt4.append(xtile)
                    for cc in range(7):
                        ps = pst.tile([P, 512], F32, tag="tps")
                        for k in range(4):
                            nc.tensor.transpose(
                                ps[:, ts(k, P)], xt4[k][:, ts(cc, P)], ident[:]
                            )
                        nc.vector.tensor_copy(xT[:, cc, ts(tcg, 512)], ps[:])

            # ---------------- phase 1: projections ----------------
            with tc.tile_pool(name="w1", bufs=1) as w1, \
                 tc.tile_pool(name="vtt", bufs=2) as vtt, \
                 tc.tile_pool(name="pst2", bufs=2, space="PSUM") as pst2, \
                 tc.tile_pool(name="psp", bufs=2, space="PSUM") as psp:
                WqH = w1.tile([P, 7, DH], F32R, tag="WqH")
                nc.sync.dma_start(
                    WqH[:], d["wq"].rearrange("(co ci) n -> ci co n", ci=P)
                )
                WkR = w1.tile([P, 7, GL, P], F32R, tag="WkR")
                wkv = d["wk"].rearrange("(co ci) n -> ci co n", ci=P)
                for g in range(GL):
                    for i in range(4):
                        nc.sync.dma_start(
                            WkR[:, :, g, ts(i, D)], wkv[:, :, ts(g, D)]
                        )
                WvH = w1.tile([P, 7, DKV], F32R, tag="WvH")
                nc.sync.dma_start(
                    WvH[:], d["wv"].rearrange("(co ci) n -> ci co n", ci=P)
                )

                # QT: out[m=dim chunk, n=t] accumulate over C chunks
                for mc in range(4):
                    M = P if mc < 3 else 64
                    for nk in range(4):
                        ps = psp.tile([P, 512], F32, tag="pps")
                        for c in range(7):
                            nc.tensor.matmul(
                                ps[:M, :],
                                lhsT=WqH[:, c, mc * P : mc * P + M],
                                rhs=xT[:, c, ts(nk, 512)],
                                start=(c == 0),
                                stop=(c == 6),
                            )
                        nc.vector.tensor_copy(QT[:M, mc, ts(nk, 512)], ps[:M, :])
                # KT (replicated): per local group
                for g in range(GL):
                    for nk in range(4):
                        ps = psp.tile([P, 512], F32, tag="pps")
                        for c in range(7):
                            nc.tensor.matmul(
                                ps[:],
                                lhsT=WkR[:, c, g, :],
                                rhs=xT[:, c, ts(nk, 512)],
                                start=(c == 0),
                                stop=(c == 6),
                            )
                        nc.vector.tensor_copy(KT[:, g, ts(nk, 512)], ps[:])
                # VT then transpose to V
                for nk in range(4):
                    ps = psp.tile([P, 512], F32, tag="pps")
                    for c in range(7):
                        nc.tensor.matmul(
                            ps[:],
                            lhsT=WvH[:, c, :],
                            rhs=xT[:, c, ts(nk, 512)],
                            start=(c == 0),
                            stop=(c == 6),
                        )
                    vts = vtt.tile([P, 512], F32, tag="vts")
                    nc.vector.tensor_copy(vts[:], ps[:])
                    for k in range(4):
                        vps = pst2.tile([P, 512], F32, tag="tps")
                        nc.tensor.transpose(vps[:, :P], vts[:, ts(k, P)], ident[:])
                        nc.vector.tensor_copy(
                            V[:, nk * 4 + k, :, 0:D],
                            vps[:, :P].rearrange("p (g e) -> p g e", g=GL),
                        )

        # ---------------- phase 2+3: attention + out-proj ----------------
        with tc.tile_pool(name="w2", bufs=1) as w2, \
             tc.tile_pool(name="pts", bufs=2) as pts, \
             tc.tile_pool(name="ziP", bufs=2) as zip_, \
             tc.tile_pool(name="outs", bufs=2) as outs_p, \
             tc.tile_pool(name="pss", bufs=2, space="PSUM") as pss, \
             tc.tile_pool(name="psav", bufs=2, space="PSUM") as psav:
            AOT = w2.tile([P, 4, T], F32R, tag="AOT")
            WoH = w2.tile([P, 4, C], F32R, tag="WoH")
            nc.sync.dma_start(
                WoH[:, :3, :], d["wo"][: 3 * P, :].rearrange("(co ci) n -> ci co n", ci=P)
            )
            nc.sync.dma_start(WoH[:64, 3, :], d["wo"][3 * P :, :])
            ov = d["out"].rearrange("(to ti) c -> ti to c", ti=P)

            for qc in range(4):
                qs = qc * 512
                for pr in range(7):
                    h0 = 2 * pr
                    g = h0 // 4
                    j0 = h0 % 4
                    avt = psav.tile([64, 2, 512], F32, tag="av")
                    nks = qs // P + 4
                    for ki in range(nks):
                        ks = ki * P
                        qoff = max(0, ks - qs)
                        pt = pts.tile([P, 2, 512], F32R, tag="pt")
                        sp = pss.tile([P, 2, 512], F32, tag="sp")
                        for j2 in range(2):
                            j = j0 + j2
                            nc.tensor.matmul(
                                sp[:, j2, qoff:512],
                                lhsT=KT[ts(j, D), g, ks : ks + P],
                                rhs=QT[ts(j, D), g, qs + qoff : qs + 512],
                                start=True,
                                stop=True,
                                tile_position=(j * D, 0),
                            )
                        nc.scalar.activation(
                            pt[:, :, qoff:512],
                            sp[:, :, qoff:512],
                            EXP,
                            scale=SCALE,
                        )
                        if ks >= qs:  # diagonal chunk: zero the triangle
                            nc.vector.tensor_tensor(
                                pt[:, :, qoff : qoff + P],
                                pt[:, :, qoff : qoff + P],
                                maskb[:, None, :].to_broadcast((P, 2, P)),
                                mybir.AluOpType.mult,
                            )
                        for j2 in range(2):
                            nc.tensor.matmul(
                                avt[0:64, j2, qoff:512],
                                lhsT=V[:, ki, g, 0:64],
                                rhs=pt[:, j2, qoff:512],
                                start=(ki == 0),
                                stop=(ki == nks - 1),
                                skip_group_check=True,
                            )
                    zq = pss.tile([P, 2, 512], F32, tag="sp")
                    for j2 in range(2):
                        h = h0 + j2
                        av = avt[:, j2, :]
                        zt = zip_.tile([64, 512], F32R, tag="zt")
                        nc.vector.tensor_copy(zt[D:64, :], av[D:64, :])
                        nc.tensor.matmul(
                            zq[0:D, j2, :],
                            lhsT=identr[D:64, D:64],
                            rhs=zt[D:64, :],
                            start=True,
                            stop=True,
                            tile_position=(D, 0),
                        )
                        zs = zip_.tile([D, 512], F32, tag="zs")
                        nc.vector.reciprocal_approx_fast(zs[:], zq[0:D, j2, :])
                        ao = zip_.tile([D, 512], F32R, tag="ao")
                        nc.vector.tensor_tensor(
                            ao[:],
                            av[0:D, :],
                            zs[:],
                            mybir.AluOpType.mult,
                        )
                        nc.sync.dma_start(
                            AOT[ts(h % 4, D), g, qs : qs + 512], ao[:]
                        )
                # out-projection for this q-chunk
                for tcl in range(4):
                    tg = qc * 4 + tcl
                    ob = outs_p.tile([P, C], F32, tag="ob")
                    for ncol in range(2):
                        pot = psav.tile([P, 2, 512], F32, tag="av")
                        po = pot[:, 0, 0:448]
                        for c in range(4):
                            K = P if c < 3 else 64
                            nc.tensor.matmul(
                                po,
                                lhsT=AOT[:K, c, qs + tcl * P : qs + (tcl + 1) * P],
                                rhs=WoH[:K, c, ncol * 448 : (ncol + 1) * 448],
                                start=(c == 0),
                                stop=(c == 3),
                            )
                        nc.vector.tensor_copy(ob[:, ncol * 448 : (ncol + 1) * 448], po)
                    nc.sync.dma_start(ov[:, tg, :], ob[:])


_NC_CACHE = None


def _build():
    global _NC_CACHE
    if _NC_CACHE is not None:
        return _NC_CACHE
    nc = bacc.Bacc("TRN2", target_bir_lowering=False, debug=False, num_devices=8)
    d = {
        "x": nc.dram_tensor("x", (T, C), F32, kind="ExternalInput"),
        "wq": nc.dram_tensor("wq", (C, DH), F32R, kind="ExternalInput"),
        "wk": nc.dram_tensor("wk", (C, DKV), F32R, kind="ExternalInput"),
        "wv": nc.dram_tensor("wv", (C, DKV), F32R, kind="ExternalInput"),
        "wo": nc.dram_tensor("wo", (DH, C), F32R, kind="ExternalInput"),
        "ident": nc.dram_tensor("ident", (P, P), F32, kind="ExternalInput"),
        "mask": nc.dram_tensor("mask", (P, P), F32, kind="ExternalInput"),
        "vones": nc.dram_tensor("vones", (P, 16 * GL * D), F32R, kind="ExternalInput"),
        "identr": nc.dram_tensor("identr", (P, P), F32R, kind="ExternalInput"),
        "out": nc.dram_tensor("out", (T, C), F32, kind="ExternalOutput"),

    }
    with tile.TileContext(nc) as tc:
        _trace(tc, {k: v[:] for k, v in d.items()})
    nc.compile()
    _NC_CACHE = nc
    return nc


def _in_maps(x, Wq, Wk, Wv, Wo):
    ident = np.eye(P, dtype=np.float32)
    vones = np.ones((P, 16 * GL * D), dtype=np.float32)
    maskb = (
        np.arange(P)[None, :] >= np.arange(P)[:, None]
    ).astype(np.float32)  # [kpos_p, q_j] valid when j >= p
    maps = []
    for c in range(8):
        b, hf = c // 2, c % 2
        hcols = np.concatenate([np.arange(32 * h, 32 * h + 32) for h in HEADS_HALF[hf]])
        kcols = np.concatenate([np.arange(32 * g, 32 * g + 32) for g in KV_HALF[hf]])
        maps.append(
            {
                "x": np.ascontiguousarray(x[b]),
                "wq": np.ascontiguousarray(Wq[:, hcols]),
                "wk": np.ascontiguousarray(Wk[:, kcols]),
                "wv": np.ascontiguousarray(Wv[:, kcols]),
                "wo": np.ascontiguousarray(Wo[hcols, :]),
                "ident": ident,
                "mask": maskb,
                "vones": vones,
                "identr": ident,
            }
        )
    return maps


def run(x, Wq, Wk, Wv, Wo, trace=False):
    nc = _build()
    res = run_bass_kernel_spmd(
        nc, _in_maps(x, Wq, Wk, Wv, Wo), core_ids=list(range(8)), trace=trace
    )
    outs = [r["out"] for r in res.results]
    final = np.empty((4, T, C), np.float32)
    for b in range(4):
        final[b] = outs[2 * b] + outs[2 * b + 1]
    return final, res


def kernel(x, Wq, Wk, Wv, Wo):
    x = np.asarray(x, dtype=np.float32)
    out, _ = run(
        x,
        np.asarray(Wq, np.float32),
        np.asarray(Wk, np.float32),
        np.asarray(Wv, np.float32),
        np.asarray(Wo, np.float32),
    )
    return out
